# revision 49
# baseline (speedup 1.0000x reference)
"""Trainium2 Bass kernel for nn_Adapter (per-token candidate attention + MLP + LN).

Data-parallel over tokens across 8 NeuronCores. Matmuls run in bf16 (fp32 PSUM
accumulation); softmax / attention-combine / layernorm run in fp32 on DVE/ACT.

Per-core pipeline (Tc = T/8 tokens, t-tiles of 128):
  - x, c streamed in; cast to bf16 via SWDGE DMA; PE-transposed to K-major.
  - q/k/v projections: PE matmuls, activations-stationary, weights moving.
  - scores: DVE mult + grouped reduce (contraction d=64, per candidate).
  - masked softmax over N=8 (no max-subtract needed; scores are O(1)).
  - ctx: DVE e-weighted accumulation of v from PSUM.
  - MLP: transposed-layout bf16 matmuls (weights stationary), gelu on ACT.
  - residual + layernorm fused on DVE/ACT; output DMA in natural layout.
"""

import numpy as np
import ml_dtypes

import concourse.bass as bass
import concourse.mybir as mybir
import concourse.tile as tile
from concourse.bass_utils import run_bass_kernel_spmd
from concourse.masks import make_identity

F32 = mybir.dt.float32
BF16 = mybir.dt.bfloat16
AX = mybir.AxisListType.X
ALU = mybir.AluOpType
ACTF = mybir.ActivationFunctionType

HID = 768
NH = 12
HD = 64
NCAND = 8
NCORES = 8
EPS = 1e-12
NI = HID // 128          # 6 input-feature chunks
NJ4 = 4 * HID // 128     # 24 hidden chunks
ESCALE = 1.0 / np.sqrt(HD)

_CACHE = {}


def _split_excess_waits(nc, max_waits=1):
    """walrus in this container only packs ONE sync-wait per ISA instruction;
    move excess on_wait entries onto inserted same-engine Drain carriers."""
    for func in nc.m.functions:
        for block in func.blocks:
            new = []
            for inst in block.instructions:
                si = inst.sync_info
                if si is not None:
                    waits = list(si.on_wait)
                    if len(waits) > max_waits:
                        k = 0
                        while len(waits) > max_waits:
                            chunk, waits = waits[:max_waits], waits[max_waits:]
                            carrier = mybir.InstNoOp(
                                name=f"{inst.name}-ws{k}", engine=inst.engine,
                                sync_info=mybir.SyncInfo(on_wait=chunk,
                                                         on_update=[]))
                            nc.register_instruction(carrier, overwrite=True)
                            new.append(carrier)
                            k += 1
                        inst.sync_info = mybir.SyncInfo(
                            on_wait=waits, on_update=list(si.on_update))
                new.append(inst)
            block.instructions = new


def build(tc_tokens, has_b, has_aff):
    """Build the single-core Bass graph (same graph runs SPMD on all cores)."""
    nt = tc_tokens // 128
    nc = bass.Bass()

    x_d = nc.dram_tensor("x", [tc_tokens, HID], F32, kind="ExternalInput")
    c_d = nc.dram_tensor("c", [tc_tokens, NCAND, HID], F32, kind="ExternalInput")
    m_d = nc.dram_tensor("m", [tc_tokens, NCAND], F32, kind="ExternalInput")
    wq_d = nc.dram_tensor("wq", [HID, HID], BF16, kind="ExternalInput")
    wk_d = nc.dram_tensor("wk", [HID, HID], BF16, kind="ExternalInput")
    wv_d = nc.dram_tensor("wv", [HID, HID], BF16, kind="ExternalInput")
    wt_d = nc.dram_tensor("wt", [HID, 4 * HID], BF16, kind="ExternalInput")
    wc_d = nc.dram_tensor("wc", [4 * HID, HID], BF16, kind="ExternalInput")
    if has_b:
        bq_d = nc.dram_tensor("bq", [HID], F32, kind="ExternalInput")
        bk_d = nc.dram_tensor("bk", [HID], F32, kind="ExternalInput")
        bv_d = nc.dram_tensor("bv", [HID], F32, kind="ExternalInput")
        bt_d = nc.dram_tensor("bt", [4 * HID], F32, kind="ExternalInput")
        bc_d = nc.dram_tensor("bc", [HID], F32, kind="ExternalInput")
    if has_aff:
        ga_d = nc.dram_tensor("ga", [HID], F32, kind="ExternalInput")
        be_d = nc.dram_tensor("be", [HID], F32, kind="ExternalInput")
    o_d = nc.dram_tensor("out", [tc_tokens, HID], F32, kind="ExternalOutput")

    with tile.TileContext(nc) as tc:
        consts = tc.alloc_tile_pool(name="consts", bufs=1)
        wpool = tc.alloc_tile_pool(name="wpool", bufs=1)
        mlpw = tc.alloc_tile_pool(name="mlpw", bufs=3)
        stage = tc.alloc_tile_pool(name="stage", bufs=6)
        xp = tc.alloc_tile_pool(name="xp", bufs=2)
        ctp = tc.alloc_tile_pool(name="ctp", bufs=2)
        qp = tc.alloc_tile_pool(name="qp", bufs=2)
        attn = tc.alloc_tile_pool(name="attn", bufs=2)
        sm = tc.alloc_tile_pool(name="sm", bufs=4)
        chk = tc.alloc_tile_pool(name="chk", bufs=2)
        h1p = tc.alloc_tile_pool(name="h1p", bufs=1)
        lnp = tc.alloc_tile_pool(name="lnp", bufs=2)

        ps_tr = tc.alloc_tile_pool(name="ps_tr", bufs=1, space="PSUM")
        ps_big = tc.alloc_tile_pool(name="ps_big", bufs=2, space="PSUM")
        ps_mlp = tc.alloc_tile_pool(name="ps_mlp", bufs=3, space="PSUM")

        ident_b = consts.tile([128, 128], BF16)
        make_identity(nc, ident_b)
        ident_f = consts.tile([128, 128], F32)
        make_identity(nc, ident_f)
        c1e10 = consts.tile([128, 1], F32)
        nc.vector.memset(c1e10, 1e-10)
        ceps = consts.tile([128, 1], F32)
        nc.vector.memset(ceps, EPS)

        # resident qkv weights, pre-transposed on host: [in_i, out_j]
        wq_sb = wpool.tile([128, NI, HID], BF16)
        nc.sync.dma_start(out=wq_sb, in_=wq_d.rearrange("(c p) j -> p c j", p=128))
        wk_sb = wpool.tile([128, NI, HID], BF16)
        nc.sync.dma_start(out=wk_sb, in_=wk_d.rearrange("(c p) j -> p c j", p=128))
        wv_sb = wpool.tile([128, NI, HID], BF16)
        nc.sync.dma_start(out=wv_sb, in_=wv_d.rearrange("(c p) j -> p c j", p=128))

        if has_b:
            bq_rep = consts.tile([128, HID], F32)
            nc.gpsimd.dma_start(out=bq_rep, in_=bq_d.to_broadcast([128, HID]))
            bk_rep = consts.tile([128, HID], F32)
            nc.gpsimd.dma_start(out=bk_rep, in_=bk_d.to_broadcast([128, HID]))
            bv_rep = consts.tile([128, HID], F32)
            nc.gpsimd.dma_start(out=bv_rep, in_=bv_d.to_broadcast([128, HID]))
            bt_sb = consts.tile([128, NJ4], F32)
            nc.sync.dma_start(out=bt_sb, in_=bt_d.rearrange("(c p) -> p c", p=128))
            bc_sb = consts.tile([128, NI], F32)
            nc.sync.dma_start(out=bc_sb, in_=bc_d.rearrange("(c p) -> p c", p=128))
        if has_aff:
            ga_rep = consts.tile([128, HID], F32)
            nc.gpsimd.dma_start(out=ga_rep, in_=ga_d.to_broadcast([128, HID]))
            be_rep = consts.tile([128, HID], F32)
            nc.gpsimd.dma_start(out=be_rep, in_=be_d.to_broadcast([128, HID]))

        drp = tc.alloc_tile_pool(name="drp", bufs=12, space="DRAM")

        # chunking for the MLP (groups of up to 4 t-tiles -> 512-token chunks)
        chunks = [list(range(s, min(s + 4, nt))) for s in range(0, nt, 4)]

        for chunk in chunks:
            cw = 128 * len(chunk)
            ctxT = chk.tile([128, NI, 512], BF16, tag="ctxT")
            ctx_bs = []

            for tloc, tt in enumerate(chunk):
                t0 = tt * 128
                # ---- load + transpose x (bf16, K-major) ----
                x_b = xp.tile([128, HID], BF16, tag="x_b")
                nc.gpsimd.dma_start(out=x_b, in_=x_d[t0:t0 + 128, :])  # cast f32->bf16
                m_t = xp.tile([128, NCAND], F32, tag="m_t")
                nc.sync.dma_start(out=m_t, in_=m_d[t0:t0 + 128, :])

                xT_ps = ps_tr.tile([128, HID], BF16, tag="trps")
                for i in range(NI):
                    nc.tensor.transpose(xT_ps[:, i * 128:(i + 1) * 128],
                                        x_b[:, i * 128:(i + 1) * 128], ident_b)
                xT = xp.tile([128, NI, 128], BF16, tag="xT")
                nc.scalar.copy(xT, xT_ps.rearrange("p (c j) -> p c j", c=NI))

                # ---- q projection ----
                q_ps = ps_big.tile([128, HID], F32, tag="big")
                for i in range(NI):
                    nc.tensor.matmul(q_ps[:, :512], xT[:, i, :], wq_sb[:, i, :512],
                                     start=(i == 0), stop=(i == NI - 1))
                    nc.tensor.matmul(q_ps[:, 512:], xT[:, i, :], wq_sb[:, i, 512:],
                                     start=(i == 0), stop=(i == NI - 1))
                q_sb = qp.tile([128, HID], BF16, tag="q_sb")
                if has_b:
                    q_f = qp.tile([128, HID], F32, tag="q_f")
                    nc.scalar.copy(q_f, q_ps)
                    nc.vector.tensor_add(q_sb, q_f, bq_rep)
                else:
                    nc.scalar.copy(q_sb, q_ps)

                # ---- load c slices: cast to a bf16 DRAM bounce, then K-major
                # via the DMA xbar transpose (no PE/ACT involvement) ----
                cT = ctp.tile([128, NCAND * NI, 128], BF16, tag="cT")
                for n in range(NCAND):
                    c_bf = drp.tile([128, HID], BF16, tag="c_bf")
                    nc.gpsimd.dma_start(out=c_bf, in_=c_d[t0:t0 + 128, n, :])
                    for i in range(NI):
                        nc.sync.dma_start_transpose(
                            out=cT[:, n * NI + i, :],
                            in_=c_bf[:, i * 128:(i + 1) * 128])

                # ---- scores: k_n projection + q.k reduce ----
                scores = attn.tile([128, NH, NCAND], F32, tag="scores")
                for n in range(NCAND):
                    k_ps = ps_big.tile([128, HID], F32, tag="big")
                    for i in range(NI):
                        lhsT = cT[:, n * NI + i, :]
                        nc.tensor.matmul(k_ps[:, :512], lhsT, wk_sb[:, i, :512],
                                         start=(i == 0), stop=(i == NI - 1))
                        nc.tensor.matmul(k_ps[:, 512:], lhsT, wk_sb[:, i, 512:],
                                         start=(i == 0), stop=(i == NI - 1))
                    prod = attn.tile([128, HID], F32, tag="prod")
                    nc.vector.tensor_mul(prod, k_ps, q_sb)
                    nc.vector.tensor_reduce(
                        out=scores[:, :, n:n + 1],
                        in_=prod.rearrange("p (h d) -> p h d", h=NH),
                        axis=AX, op=ALU.add)
                if has_b:
                    # scores += sum_d q[t,h,d]*bk[h,d]  (constant across n)
                    prod = attn.tile([128, HID], F32, tag="prod")
                    nc.vector.tensor_mul(prod, q_sb, bk_rep)
                    qbk = sm.tile([128, NH, 1], F32, tag="qbk")
                    nc.vector.tensor_reduce(
                        out=qbk, in_=prod.rearrange("p (h d) -> p h d", h=NH),
                        axis=AX, op=ALU.add)
                    nc.vector.tensor_add(scores, scores,
                                         qbk.broadcast_to([128, NH, NCAND]))

                # ---- masked softmax over n (no max-subtract; scores are O(1)) ----
                # masked scores -> 0 (exp(0)=1.0 == exp(1e-10*scale) in fp32)
                om = sm.tile([128, NCAND], F32, tag="om")
                nc.vector.tensor_scalar(out=om, in0=m_t, scalar1=-1.0, scalar2=1.0,
                                        op0=ALU.mult, op1=ALU.add)
                nc.vector.tensor_mul(scores, scores,
                                     om.unsqueeze(1).broadcast_to([128, NH, NCAND]))
                e_t = attn.tile([128, NH, NCAND], F32, tag="e_t")
                nc.scalar.activation(e_t, scores, ACTF.Exp, scale=ESCALE)
                esum = sm.tile([128, NH], F32, tag="esum")
                nc.vector.tensor_reduce(out=esum, in_=e_t, axis=AX, op=ALU.add)
                recip = sm.tile([128, NH], F32, tag="recip")
                nc.vector.reciprocal(recip, esum)
                msum = sm.tile([128, 1], F32, tag="msum")
                nc.vector.tensor_reduce(out=msum, in_=m_t, axis=AX, op=ALU.add)
                notall = sm.tile([128, 1], F32, tag="notall")
                nc.vector.tensor_scalar(out=notall, in0=msum, scalar1=float(NCAND) - 0.5,
                                        scalar2=None, op0=ALU.is_lt)
                nc.vector.tensor_scalar(out=recip, in0=recip, scalar1=notall,
                                        scalar2=None, op0=ALU.mult)
                # pre-scale e by 1/esum (and the all-masked zeroing) so the
                # v-side accumulation needs no final normalization pass
                nc.vector.tensor_mul(e_t, e_t,
                                     recip.unsqueeze(2).broadcast_to([128, NH, NCAND]))

                # ---- ctx: v_n projection + attn-weighted accumulation ----
                ctx = attn.tile([128, NH, HD], F32, tag="ctx")
                ctx_b = attn.tile([128, HID], BF16, tag="ctx_b", bufs=5)
                ctx_b3 = ctx_b.rearrange("p (h d) -> p h d", h=NH)
                for n in range(NCAND):
                    v_ps = ps_big.tile([128, HID], F32, tag="big")
                    for i in range(NI):
                        lhsT = cT[:, n * NI + i, :]
                        nc.tensor.matmul(v_ps[:, :512], lhsT, wv_sb[:, i, :512],
                                         start=(i == 0), stop=(i == NI - 1))
                        nc.tensor.matmul(v_ps[:, 512:], lhsT, wv_sb[:, i, 512:],
                                         start=(i == 0), stop=(i == NI - 1))
                    e_b = e_t[:, :, n:n + 1].broadcast_to([128, NH, HD])
                    v3 = v_ps.rearrange("p (h d) -> p h d", h=NH)
                    if n == 0:
                        nc.vector.tensor_mul(ctx, v3, e_b)
                    else:
                        prodv = attn.tile([128, NH, HD], F32, tag="prodv")
                        nc.vector.tensor_mul(prodv, v3, e_b)
                        last = (n == NCAND - 1) and not has_b
                        nc.vector.tensor_add(ctx_b3 if last else ctx, ctx, prodv)
                if has_b:
                    # ctx += bv * notall (softmax weights sum to 1)
                    nc.vector.scalar_tensor_tensor(
                        out=ctx_b3, in0=bv_rep.rearrange("p (h d) -> p h d", h=NH),
                        scalar=notall, in1=ctx, op0=ALU.mult, op1=ALU.add)
                ctx_bs.append(ctx_b)

            # ---- transpose ctx tiles into the chunk's K-major buffer ----
            for tloc in range(len(chunk)):
                ctxT_ps = ps_tr.tile([128, HID], BF16, tag="trps")
                for i in range(NI):
                    nc.tensor.transpose(ctxT_ps[:, i * 128:(i + 1) * 128],
                                        ctx_bs[tloc][:, i * 128:(i + 1) * 128],
                                        ident_b)
                nc.scalar.copy(
                    ctxT[:, :, tloc * 128:(tloc + 1) * 128],
                    ctxT_ps.rearrange("p (c j) -> p c j", c=NI))

            # ---- MLP over the chunk (transposed layout) ----
            h1T = h1p.tile([128, NJ4, 512], BF16, tag="h1T")
            for j in range(NJ4):
                wt_t = mlpw.tile([128, NI, 128], BF16, tag="wt_t", bufs=6)
                nc.sync.dma_start(
                    out=wt_t,
                    in_=wt_d[:, j * 128:(j + 1) * 128].rearrange(
                        "(c p) j -> p c j", p=128))
                h1_ps = ps_mlp.tile([128, 512], F32, tag="mlpps")
                for i in range(NI):
                    nc.tensor.matmul(h1_ps[:, :cw], wt_t[:, i, :], ctxT[:, i, :cw],
                                     start=(i == 0), stop=(i == NI - 1))
                nc.scalar.activation(h1T[:, j, :cw], h1_ps[:, :cw], ACTF.Gelu,
                                     bias=(bt_sb[:, j:j + 1] if has_b else 0.0))

            o2T = chk.tile([128, NI, 512], F32, tag="o2T")
            for o in range(NI):
                wc_t = mlpw.tile([128, NJ4, 128], BF16, tag="wc_t")
                nc.sync.dma_start(
                    out=wc_t,
                    in_=wc_d[:, o * 128:(o + 1) * 128].rearrange(
                        "(c p) j -> p c j", p=128))
                o2_ps = ps_mlp.tile([128, 512], F32, tag="mlpps")
                for j in range(NJ4):
                    nc.tensor.matmul(o2_ps[:, :cw], wc_t[:, j, :], h1T[:, j, :cw],
                                     start=(j == 0), stop=(j == NJ4 - 1))
                nc.scalar.activation(o2T[:, o, :cw], o2_ps[:, :cw], ACTF.Copy,
                                     bias=(bc_sb[:, o:o + 1] if has_b else 0.0))

            # ---- back to natural layout + residual + layernorm ----
            for tloc, tt in enumerate(chunk):
                t0 = tt * 128
                o2n_ps = ps_big.tile([128, HID], F32, tag="big")
                for o in range(NI):
                    nc.tensor.transpose(o2n_ps[:, o * 128:(o + 1) * 128],
                                        o2T[:, o, tloc * 128:(tloc + 1) * 128],
                                        ident_f)
                x_f = lnp.tile([128, HID], F32, tag="x_f")
                nc.sync.dma_start(out=x_f, in_=x_d[t0:t0 + 128, :])

                y_sb = lnp.tile([128, HID], F32, tag="y_sb")
                sums = sm.tile([128, 1], F32, tag="sums")
                nc.vector.scalar_tensor_tensor(
                    out=y_sb, in0=o2n_ps, scalar=1.0, in1=x_f,
                    op0=ALU.mult, op1=ALU.add, accum_out=sums)
                out_sb = lnp.tile([128, HID], F32, tag="out_sb")
                sumsq = sm.tile([128, 1], F32, tag="sumsq")
                nc.vector.scalar_tensor_tensor(
                    out=out_sb, in0=y_sb, scalar=1.0, in1=y_sb,
                    op0=ALU.mult, op1=ALU.mult, accum_out=sumsq)
                mean = sm.tile([128, 1], F32, tag="mean")
                nc.vector.tensor_scalar(out=mean, in0=sums, scalar1=1.0 / HID,
                                        scalar2=None, op0=ALU.mult)
                msq = sm.tile([128, 1], F32, tag="msq")
                nc.vector.tensor_mul(msq, mean, mean)
                var = sm.tile([128, 1], F32, tag="var")
                nc.vector.tensor_scalar(out=var, in0=sumsq, scalar1=1.0 / HID,
                                        scalar2=msq, op0=ALU.mult, op1=ALU.subtract)
                # rstd = exp(-0.5 * ln(var + eps)) — Ln/Exp share one ACT table set
                lnv = sm.tile([128, 1], F32, tag="lnv")
                nc.scalar.activation(lnv, var, ACTF.Ln, bias=ceps)
                rstd = sm.tile([128, 1], F32, tag="rstd")
                nc.scalar.activation(rstd, lnv, ACTF.Exp, scale=-0.5)

                nc.vector.tensor_scalar(out=out_sb, in0=y_sb, scalar1=mean,
                                        scalar2=rstd, op0=ALU.subtract, op1=ALU.mult)
                if has_aff:
                    nc.vector.tensor_mul(out_sb, out_sb, ga_rep)
                    nc.vector.tensor_add(out_sb, out_sb, be_rep)
                nc.sync.dma_start(out=o_d[t0:t0 + 128, :], in_=out_sb)

        for p in reversed((consts, wpool, mlpw, stage, xp, ctp, qp, attn, sm,
                           chk, h1p, lnp, ps_tr, ps_big, ps_mlp)):
            p.release()
    _split_excess_waits(nc)
    return nc


def _prep(inputs):
    ins = {k: np.asarray(v) for k, v in inputs.items()}
    x = ins["layer_output"].astype(np.float32)
    c = ins["candidates_embeddings"].astype(np.float32)
    m = ins["candidates_mask"].astype(np.float32)
    B, S, H = x.shape
    T = B * S
    n_ = c.shape[2]
    assert H == HID and n_ == NCAND and T % (NCORES * 128) == 0

    has_b = any(np.any(ins[k] != 0) for k in ("bq", "bk", "bv", "bt", "bc"))
    has_aff = bool(np.any(ins["gamma"] != 1) or np.any(ins["beta"] != 0))

    bf = ml_dtypes.bfloat16
    weights = {
        "wq": np.ascontiguousarray(ins["Wq"].astype(np.float32).T).astype(bf),
        "wk": np.ascontiguousarray(ins["Wk"].astype(np.float32).T).astype(bf),
        "wv": np.ascontiguousarray(ins["Wv"].astype(np.float32).T).astype(bf),
        "wt": np.ascontiguousarray(ins["Wt"].astype(np.float32).T).astype(bf),
        "wc": np.ascontiguousarray(ins["Wc"].astype(np.float32).T).astype(bf),
    }
    if has_b:
        for k_src, k_dst in (("bq", "bq"), ("bk", "bk"), ("bv", "bv"),
                             ("bt", "bt"), ("bc", "bc")):
            weights[k_dst] = ins[k_src].astype(np.float32)
    if has_aff:
        weights["ga"] = ins["gamma"].astype(np.float32)
        weights["be"] = ins["beta"].astype(np.float32)

    tc_tokens = T // NCORES
    xf = x.reshape(T, H)
    cf = c.reshape(T, NCAND, H)
    mf = m.reshape(T, NCAND)
    in_maps = []
    for k in range(NCORES):
        sl = slice(k * tc_tokens, (k + 1) * tc_tokens)
        im = {"x": np.ascontiguousarray(xf[sl]),
              "c": np.ascontiguousarray(cf[sl]),
              "m": np.ascontiguousarray(mf[sl])}
        im.update(weights)
        in_maps.append(im)
    return in_maps, tc_tokens, has_b, has_aff, (B, S, H)


def kernel(**inputs):
    in_maps, tc_tokens, has_b, has_aff, (B, S, H) = _prep(inputs)
    key = (tc_tokens, has_b, has_aff)
    if key not in _CACHE:
        _CACHE[key] = build(*key)
    nc = _CACHE[key]
    res = run_bass_kernel_spmd(nc, in_maps, core_ids=list(range(NCORES)))
    out = np.concatenate([res.results[i]["out"] for i in range(NCORES)], axis=0)
    return out.reshape(B, S, H).astype(np.float32)


# exposed for test.py profiling
def kernel_profiled(**inputs):
    in_maps, tc_tokens, has_b, has_aff, (B, S, H) = _prep(inputs)
    key = (tc_tokens, has_b, has_aff)
    if key not in _CACHE:
        _CACHE[key] = build(*key)
    nc = _CACHE[key]
    res = run_bass_kernel_spmd(nc, in_maps, core_ids=list(range(NCORES)),
                               trace=True)
    out = np.concatenate([res.results[i]["out"] for i in range(NCORES)], axis=0)
    return out.reshape(B, S, H).astype(np.float32), res


# revision 54
# speedup vs baseline: 1.0332x; 1.0332x over previous
"""Trainium2 Bass kernel for nn_Adapter (per-token candidate attention + MLP + LN).

Data-parallel over tokens across 8 NeuronCores. Matmuls run in bf16 (fp32 PSUM
accumulation); softmax / attention-combine / layernorm run in fp32 on DVE/ACT.

Per-core pipeline (Tc = T/8 tokens, t-tiles of 128):
  - x, c streamed in; cast to bf16 via SWDGE DMA; PE-transposed to K-major.
  - q/k/v projections: PE matmuls, activations-stationary, weights moving.
  - scores: DVE mult + grouped reduce (contraction d=64, per candidate).
  - masked softmax over N=8 (no max-subtract needed; scores are O(1)).
  - ctx: DVE e-weighted accumulation of v from PSUM.
  - MLP: transposed-layout bf16 matmuls (weights stationary), gelu on ACT.
  - residual + layernorm fused on DVE/ACT; output DMA in natural layout.
"""

import numpy as np
import ml_dtypes

import concourse.bass as bass
import concourse.mybir as mybir
import concourse.tile as tile
from concourse.bass_utils import run_bass_kernel_spmd
from concourse.masks import make_identity

F32 = mybir.dt.float32
BF16 = mybir.dt.bfloat16
AX = mybir.AxisListType.X
ALU = mybir.AluOpType
ACTF = mybir.ActivationFunctionType

HID = 768
NH = 12
HD = 64
NCAND = 8
NCORES = 8
EPS = 1e-12
NI = HID // 128          # 6 input-feature chunks
NJ4 = 4 * HID // 128     # 24 hidden chunks
ESCALE = 1.0 / np.sqrt(HD)

_CACHE = {}


def _split_excess_waits(nc, max_waits=1):
    """walrus in this container only packs ONE sync-wait per ISA instruction;
    move excess on_wait entries onto inserted same-engine Drain carriers."""
    for func in nc.m.functions:
        for block in func.blocks:
            new = []
            for inst in block.instructions:
                si = inst.sync_info
                if si is not None:
                    waits = list(si.on_wait)
                    if len(waits) > max_waits:
                        k = 0
                        while len(waits) > max_waits:
                            chunk, waits = waits[:max_waits], waits[max_waits:]
                            carrier = mybir.InstNoOp(
                                name=f"{inst.name}-ws{k}", engine=inst.engine,
                                sync_info=mybir.SyncInfo(on_wait=chunk,
                                                         on_update=[]))
                            nc.register_instruction(carrier, overwrite=True)
                            new.append(carrier)
                            k += 1
                        inst.sync_info = mybir.SyncInfo(
                            on_wait=waits, on_update=list(si.on_update))
                new.append(inst)
            block.instructions = new


def build(tc_tokens, has_b, has_aff):
    """Build the single-core Bass graph (same graph runs SPMD on all cores)."""
    nt = tc_tokens // 128
    nc = bass.Bass()

    id_d = nc.dram_tensor("idb", [128, 128], BF16, kind="ExternalInput")
    idf_d = nc.dram_tensor("idf", [128, 128], F32, kind="ExternalInput")
    x_d = nc.dram_tensor("x", [tc_tokens, HID], F32, kind="ExternalInput")
    c_d = nc.dram_tensor("c", [tc_tokens, NCAND, HID], F32, kind="ExternalInput")
    m_d = nc.dram_tensor("m", [tc_tokens, NCAND], F32, kind="ExternalInput")
    wq_d = nc.dram_tensor("wq", [HID, HID], BF16, kind="ExternalInput")
    wk_d = nc.dram_tensor("wk", [HID, HID], BF16, kind="ExternalInput")
    wv_d = nc.dram_tensor("wv", [HID, HID], BF16, kind="ExternalInput")
    wt_d = nc.dram_tensor("wt", [HID, 4 * HID], BF16, kind="ExternalInput")
    wc_d = nc.dram_tensor("wc", [4 * HID, HID], BF16, kind="ExternalInput")
    if has_b:
        bq_d = nc.dram_tensor("bq", [HID], F32, kind="ExternalInput")
        bk_d = nc.dram_tensor("bk", [HID], F32, kind="ExternalInput")
        bv_d = nc.dram_tensor("bv", [HID], F32, kind="ExternalInput")
        bt_d = nc.dram_tensor("bt", [4 * HID], F32, kind="ExternalInput")
        bc_d = nc.dram_tensor("bc", [HID], F32, kind="ExternalInput")
    if has_aff:
        ga_d = nc.dram_tensor("ga", [HID], F32, kind="ExternalInput")
        be_d = nc.dram_tensor("be", [HID], F32, kind="ExternalInput")
    o_d = nc.dram_tensor("out", [tc_tokens, HID], F32, kind="ExternalOutput")

    with tile.TileContext(nc) as tc:
        consts = tc.alloc_tile_pool(name="consts", bufs=1)
        wpool = tc.alloc_tile_pool(name="wpool", bufs=1)
        mlpw = tc.alloc_tile_pool(name="mlpw", bufs=3)
        stage = tc.alloc_tile_pool(name="stage", bufs=6)
        xp = tc.alloc_tile_pool(name="xp", bufs=2)
        ctp = tc.alloc_tile_pool(name="ctp", bufs=2)
        qp = tc.alloc_tile_pool(name="qp", bufs=2)
        attn = tc.alloc_tile_pool(name="attn", bufs=2)
        sm = tc.alloc_tile_pool(name="sm", bufs=4)
        chk = tc.alloc_tile_pool(name="chk", bufs=2)
        h1p = tc.alloc_tile_pool(name="h1p", bufs=1)
        lnp = tc.alloc_tile_pool(name="lnp", bufs=2)

        ps_tr = tc.alloc_tile_pool(name="ps_tr", bufs=2, space="PSUM")
        ps_big = tc.alloc_tile_pool(name="ps_big", bufs=2, space="PSUM")
        ps_mlp = tc.alloc_tile_pool(name="ps_mlp", bufs=2, space="PSUM")

        ident_b = consts.tile([128, 128], BF16)
        nc.sync.dma_start(out=ident_b, in_=id_d[:, :])
        ident_f = consts.tile([128, 128], F32)
        nc.sync.dma_start(out=ident_f, in_=idf_d[:, :])
        m_all = consts.tile([128, nt, NCAND], F32)
        nc.sync.dma_start(out=m_all,
                          in_=m_d.rearrange("(t p) n -> p t n", p=128))
        c1e10 = consts.tile([128, 1], F32)
        nc.vector.memset(c1e10, 1e-10)
        ceps = consts.tile([128, 1], F32)
        nc.vector.memset(ceps, EPS)

        # resident qkv weights, pre-transposed on host: [in_i, out_j]
        wq_sb = wpool.tile([128, NI, HID], BF16)
        nc.sync.dma_start(out=wq_sb, in_=wq_d.rearrange("(c p) j -> p c j", p=128))
        wk_sb = wpool.tile([128, NI, HID], BF16)
        nc.sync.dma_start(out=wk_sb, in_=wk_d.rearrange("(c p) j -> p c j", p=128))
        wv_sb = wpool.tile([128, NI, HID], BF16)
        nc.sync.dma_start(out=wv_sb, in_=wv_d.rearrange("(c p) j -> p c j", p=128))

        if has_b:
            bq_rep = consts.tile([128, HID], F32)
            nc.gpsimd.dma_start(out=bq_rep, in_=bq_d.to_broadcast([128, HID]))
            bk_rep = consts.tile([128, HID], F32)
            nc.gpsimd.dma_start(out=bk_rep, in_=bk_d.to_broadcast([128, HID]))
            bv_rep = consts.tile([128, HID], F32)
            nc.gpsimd.dma_start(out=bv_rep, in_=bv_d.to_broadcast([128, HID]))
            bt_sb = consts.tile([128, NJ4], F32)
            nc.sync.dma_start(out=bt_sb, in_=bt_d.rearrange("(c p) -> p c", p=128))
            bc_sb = consts.tile([128, NI], F32)
            nc.sync.dma_start(out=bc_sb, in_=bc_d.rearrange("(c p) -> p c", p=128))
        if has_aff:
            ga_rep = consts.tile([128, HID], F32)
            nc.gpsimd.dma_start(out=ga_rep, in_=ga_d.to_broadcast([128, HID]))
            be_rep = consts.tile([128, HID], F32)
            nc.gpsimd.dma_start(out=be_rep, in_=be_d.to_broadcast([128, HID]))

        drp = tc.alloc_tile_pool(name="drp", bufs=12, space="DRAM")

        # chunking for the MLP (groups of up to 4 t-tiles -> 512-token chunks)
        chunks = [list(range(s, min(s + 4, nt))) for s in range(0, nt, 4)]

        for chunk in chunks:
            cw = 128 * len(chunk)
            ctxT = chk.tile([128, NI, 512], BF16, tag="ctxT")
            ctx_bs = []

            for tloc, tt in enumerate(chunk):
                t0 = tt * 128
                # ---- load + transpose x (bf16, K-major) ----
                x_b = xp.tile([128, HID], BF16, tag="x_b")
                nc.gpsimd.dma_start(out=x_b, in_=x_d[t0:t0 + 128, :])  # cast f32->bf16
                m_t = m_all[:, tt, :]

                xT_ps = ps_tr.tile([128, HID], BF16, tag="trps")
                for i in range(NI):
                    nc.tensor.transpose(xT_ps[:, i * 128:(i + 1) * 128],
                                        x_b[:, i * 128:(i + 1) * 128], ident_b)
                xT = xp.tile([128, NI, 128], BF16, tag="xT")
                nc.scalar.copy(xT, xT_ps.rearrange("p (c j) -> p c j", c=NI))

                # ---- q projection ----
                q_ps = ps_big.tile([128, HID], F32, tag="big")
                for i in range(NI):
                    nc.tensor.matmul(q_ps[:, :512], xT[:, i, :], wq_sb[:, i, :512],
                                     start=(i == 0), stop=(i == NI - 1))
                    nc.tensor.matmul(q_ps[:, 512:], xT[:, i, :], wq_sb[:, i, 512:],
                                     start=(i == 0), stop=(i == NI - 1))
                q_sb = qp.tile([128, HID], BF16, tag="q_sb")
                if has_b:
                    q_f = qp.tile([128, HID], F32, tag="q_f")
                    nc.scalar.copy(q_f, q_ps)
                    nc.vector.tensor_add(q_sb, q_f, bq_rep)
                else:
                    nc.scalar.copy(q_sb, q_ps)

                # ---- load c slices: cast to a bf16 DRAM bounce, then K-major
                # via the DMA xbar transpose (no PE/ACT involvement) ----
                cT = ctp.tile([128, NCAND * NI, 128], BF16, tag="cT")
                for n in range(NCAND):
                    c_bf = drp.tile([128, HID], BF16, tag="c_bf")
                    nc.gpsimd.dma_start(out=c_bf, in_=c_d[t0:t0 + 128, n, :])
                    for i in range(NI):
                        nc.sync.dma_start_transpose(
                            out=cT[:, n * NI + i, :],
                            in_=c_bf[:, i * 128:(i + 1) * 128])

                # ---- scores: k_n projection + q.k reduce ----
                scores = attn.tile([128, NH, NCAND], F32, tag="scores")
                for n in range(NCAND):
                    k_ps = ps_big.tile([128, HID], F32, tag="big")
                    for i in range(NI):
                        lhsT = cT[:, n * NI + i, :]
                        nc.tensor.matmul(k_ps[:, :512], lhsT, wk_sb[:, i, :512],
                                         start=(i == 0), stop=(i == NI - 1))
                        nc.tensor.matmul(k_ps[:, 512:], lhsT, wk_sb[:, i, 512:],
                                         start=(i == 0), stop=(i == NI - 1))
                    prod = attn.tile([128, HID], F32, tag="prod")
                    nc.vector.tensor_mul(prod, k_ps, q_sb)
                    nc.vector.tensor_reduce(
                        out=scores[:, :, n:n + 1],
                        in_=prod.rearrange("p (h d) -> p h d", h=NH),
                        axis=AX, op=ALU.add)
                if has_b:
                    # scores += sum_d q[t,h,d]*bk[h,d]  (constant across n)
                    prod = attn.tile([128, HID], F32, tag="prod")
                    nc.vector.tensor_mul(prod, q_sb, bk_rep)
                    qbk = sm.tile([128, NH, 1], F32, tag="qbk")
                    nc.vector.tensor_reduce(
                        out=qbk, in_=prod.rearrange("p (h d) -> p h d", h=NH),
                        axis=AX, op=ALU.add)
                    nc.vector.tensor_add(scores, scores,
                                         qbk.broadcast_to([128, NH, NCAND]))

                # ---- masked softmax over n (no max-subtract; scores are O(1)) ----
                # masked scores -> 0 (exp(0)=1.0 == exp(1e-10*scale) in fp32)
                om = sm.tile([128, NCAND], F32, tag="om")
                nc.vector.tensor_scalar(out=om, in0=m_t, scalar1=-1.0, scalar2=1.0,
                                        op0=ALU.mult, op1=ALU.add)
                nc.vector.tensor_mul(scores, scores,
                                     om.unsqueeze(1).broadcast_to([128, NH, NCAND]))
                e_t = attn.tile([128, NH, NCAND], F32, tag="e_t")
                nc.scalar.activation(e_t, scores, ACTF.Exp, scale=ESCALE)
                esum = sm.tile([128, NH], F32, tag="esum")
                nc.vector.tensor_reduce(out=esum, in_=e_t, axis=AX, op=ALU.add)
                recip = sm.tile([128, NH], F32, tag="recip")
                nc.vector.reciprocal(recip, esum)
                msum = sm.tile([128, 1], F32, tag="msum")
                nc.vector.tensor_reduce(out=msum, in_=m_t, axis=AX, op=ALU.add)
                notall = sm.tile([128, 1], F32, tag="notall")
                nc.vector.tensor_scalar(out=notall, in0=msum, scalar1=float(NCAND) - 0.5,
                                        scalar2=None, op0=ALU.is_lt)
                nc.vector.tensor_scalar(out=recip, in0=recip, scalar1=notall,
                                        scalar2=None, op0=ALU.mult)
                # pre-scale e by 1/esum (and the all-masked zeroing) so the
                # v-side accumulation needs no final normalization pass
                nc.vector.tensor_mul(e_t, e_t,
                                     recip.unsqueeze(2).broadcast_to([128, NH, NCAND]))

                # ---- ctx: v_n projection + attn-weighted accumulation ----
                ctx = attn.tile([128, NH, HD], F32, tag="ctx")
                ctx_b = attn.tile([128, HID], BF16, tag="ctx_b", bufs=5)
                ctx_b3 = ctx_b.rearrange("p (h d) -> p h d", h=NH)
                for n in range(NCAND):
                    v_ps = ps_big.tile([128, HID], F32, tag="big")
                    for i in range(NI):
                        lhsT = cT[:, n * NI + i, :]
                        nc.tensor.matmul(v_ps[:, :512], lhsT, wv_sb[:, i, :512],
                                         start=(i == 0), stop=(i == NI - 1))
                        nc.tensor.matmul(v_ps[:, 512:], lhsT, wv_sb[:, i, 512:],
                                         start=(i == 0), stop=(i == NI - 1))
                    e_b = e_t[:, :, n:n + 1].broadcast_to([128, NH, HD])
                    v3 = v_ps.rearrange("p (h d) -> p h d", h=NH)
                    if n == 0:
                        nc.vector.tensor_mul(ctx, v3, e_b)
                    else:
                        prodv = attn.tile([128, NH, HD], F32, tag="prodv")
                        nc.vector.tensor_mul(prodv, v3, e_b)
                        last = (n == NCAND - 1) and not has_b
                        nc.vector.tensor_add(ctx_b3 if last else ctx, ctx, prodv)
                if has_b:
                    # ctx += bv * notall (softmax weights sum to 1)
                    nc.vector.scalar_tensor_tensor(
                        out=ctx_b3, in0=bv_rep.rearrange("p (h d) -> p h d", h=NH),
                        scalar=notall, in1=ctx, op0=ALU.mult, op1=ALU.add)
                ctx_bs.append(ctx_b)

            # ---- transpose ctx tiles into the chunk's K-major buffer ----
            for tloc in range(len(chunk)):
                ctxT_ps = ps_tr.tile([128, HID], BF16, tag="trps")
                for i in range(NI):
                    nc.tensor.transpose(ctxT_ps[:, i * 128:(i + 1) * 128],
                                        ctx_bs[tloc][:, i * 128:(i + 1) * 128],
                                        ident_b)
                nc.scalar.copy(
                    ctxT[:, :, tloc * 128:(tloc + 1) * 128],
                    ctxT_ps.rearrange("p (c j) -> p c j", c=NI))

            # ---- MLP over the chunk (transposed layout) ----
            h1T = h1p.tile([128, NJ4, 512], BF16, tag="h1T")
            for j in range(NJ4):
                wt_t = mlpw.tile([128, NI, 128], BF16, tag="wt_t", bufs=6)
                nc.sync.dma_start(
                    out=wt_t,
                    in_=wt_d[:, j * 128:(j + 1) * 128].rearrange(
                        "(c p) j -> p c j", p=128))
                h1_ps = ps_mlp.tile([128, 512], F32, tag="mlpps")
                for i in range(NI):
                    nc.tensor.matmul(h1_ps[:, :cw], wt_t[:, i, :], ctxT[:, i, :cw],
                                     start=(i == 0), stop=(i == NI - 1))
                nc.scalar.activation(h1T[:, j, :cw], h1_ps[:, :cw], ACTF.Gelu,
                                     bias=(bt_sb[:, j:j + 1] if has_b else 0.0))

            o2T = chk.tile([128, NI, 512], F32, tag="o2T")
            for o in range(NI):
                wc_t = mlpw.tile([128, NJ4, 128], BF16, tag="wc_t")
                nc.sync.dma_start(
                    out=wc_t,
                    in_=wc_d[:, o * 128:(o + 1) * 128].rearrange(
                        "(c p) j -> p c j", p=128))
                o2_ps = ps_mlp.tile([128, 512], F32, tag="mlpps")
                for j in range(NJ4):
                    nc.tensor.matmul(o2_ps[:, :cw], wc_t[:, j, :], h1T[:, j, :cw],
                                     start=(j == 0), stop=(j == NJ4 - 1))
                nc.scalar.activation(o2T[:, o, :cw], o2_ps[:, :cw], ACTF.Copy,
                                     bias=(bc_sb[:, o:o + 1] if has_b else 0.0))

            # ---- back to natural layout + residual + layernorm ----
            for tloc, tt in enumerate(chunk):
                t0 = tt * 128
                o2n_ps = ps_big.tile([128, HID], F32, tag="big")
                for o in range(NI):
                    nc.tensor.transpose(o2n_ps[:, o * 128:(o + 1) * 128],
                                        o2T[:, o, tloc * 128:(tloc + 1) * 128],
                                        ident_f)
                x_f = lnp.tile([128, HID], F32, tag="x_f")
                nc.sync.dma_start(out=x_f, in_=x_d[t0:t0 + 128, :])

                y_sb = lnp.tile([128, HID], F32, tag="y_sb")
                sums = sm.tile([128, 1], F32, tag="sums")
                nc.vector.scalar_tensor_tensor(
                    out=y_sb, in0=o2n_ps, scalar=1.0, in1=x_f,
                    op0=ALU.mult, op1=ALU.add, accum_out=sums)
                out_sb = lnp.tile([128, HID], F32, tag="out_sb")
                sumsq = sm.tile([128, 1], F32, tag="sumsq")
                nc.vector.scalar_tensor_tensor(
                    out=out_sb, in0=y_sb, scalar=1.0, in1=y_sb,
                    op0=ALU.mult, op1=ALU.mult, accum_out=sumsq)
                mean = sm.tile([128, 1], F32, tag="mean")
                nc.vector.tensor_scalar(out=mean, in0=sums, scalar1=1.0 / HID,
                                        scalar2=None, op0=ALU.mult)
                msq = sm.tile([128, 1], F32, tag="msq")
                nc.vector.tensor_mul(msq, mean, mean)
                var = sm.tile([128, 1], F32, tag="var")
                nc.vector.tensor_scalar(out=var, in0=sumsq, scalar1=1.0 / HID,
                                        scalar2=msq, op0=ALU.mult, op1=ALU.subtract)
                # rstd = exp(-0.5 * ln(var + eps)) — Ln/Exp share one ACT table set
                lnv = sm.tile([128, 1], F32, tag="lnv")
                nc.scalar.activation(lnv, var, ACTF.Ln, bias=ceps)
                rstd = sm.tile([128, 1], F32, tag="rstd")
                nc.scalar.activation(rstd, lnv, ACTF.Exp, scale=-0.5)

                nc.vector.tensor_scalar(out=out_sb, in0=y_sb, scalar1=mean,
                                        scalar2=rstd, op0=ALU.subtract, op1=ALU.mult)
                if has_aff:
                    nc.vector.tensor_mul(out_sb, out_sb, ga_rep)
                    nc.vector.tensor_add(out_sb, out_sb, be_rep)
                nc.sync.dma_start(out=o_d[t0:t0 + 128, :], in_=out_sb)

        for p in reversed((consts, wpool, mlpw, stage, xp, ctp, qp, attn, sm,
                           chk, h1p, lnp, ps_tr, ps_big, ps_mlp)):
            p.release()
    _split_excess_waits(nc)
    return nc


def _prep(inputs):
    ins = {k: np.asarray(v) for k, v in inputs.items()}
    x = ins["layer_output"].astype(np.float32)
    c = ins["candidates_embeddings"].astype(np.float32)
    m = ins["candidates_mask"].astype(np.float32)
    B, S, H = x.shape
    T = B * S
    n_ = c.shape[2]
    assert H == HID and n_ == NCAND and T % (NCORES * 128) == 0

    has_b = any(np.any(ins[k] != 0) for k in ("bq", "bk", "bv", "bt", "bc"))
    has_aff = bool(np.any(ins["gamma"] != 1) or np.any(ins["beta"] != 0))

    bf = ml_dtypes.bfloat16
    weights = {
        "idb": np.eye(128, dtype=np.float32).astype(bf),
        "idf": np.eye(128, dtype=np.float32),
        "wq": np.ascontiguousarray(ins["Wq"].astype(np.float32).T).astype(bf),
        "wk": np.ascontiguousarray(ins["Wk"].astype(np.float32).T).astype(bf),
        "wv": np.ascontiguousarray(ins["Wv"].astype(np.float32).T).astype(bf),
        "wt": np.ascontiguousarray(ins["Wt"].astype(np.float32).T).astype(bf),
        "wc": np.ascontiguousarray(ins["Wc"].astype(np.float32).T).astype(bf),
    }
    if has_b:
        for k_src, k_dst in (("bq", "bq"), ("bk", "bk"), ("bv", "bv"),
                             ("bt", "bt"), ("bc", "bc")):
            weights[k_dst] = ins[k_src].astype(np.float32)
    if has_aff:
        weights["ga"] = ins["gamma"].astype(np.float32)
        weights["be"] = ins["beta"].astype(np.float32)

    tc_tokens = T // NCORES
    xf = x.reshape(T, H)
    cf = c.reshape(T, NCAND, H)
    mf = m.reshape(T, NCAND)
    in_maps = []
    for k in range(NCORES):
        sl = slice(k * tc_tokens, (k + 1) * tc_tokens)
        im = {"x": np.ascontiguousarray(xf[sl]),
              "c": np.ascontiguousarray(cf[sl]),
              "m": np.ascontiguousarray(mf[sl])}
        im.update(weights)
        in_maps.append(im)
    return in_maps, tc_tokens, has_b, has_aff, (B, S, H)


def kernel(**inputs):
    in_maps, tc_tokens, has_b, has_aff, (B, S, H) = _prep(inputs)
    key = (tc_tokens, has_b, has_aff)
    if key not in _CACHE:
        _CACHE[key] = build(*key)
    nc = _CACHE[key]
    res = run_bass_kernel_spmd(nc, in_maps, core_ids=list(range(NCORES)))
    out = np.concatenate([res.results[i]["out"] for i in range(NCORES)], axis=0)
    return out.reshape(B, S, H).astype(np.float32)


# exposed for test.py profiling
def kernel_profiled(**inputs):
    in_maps, tc_tokens, has_b, has_aff, (B, S, H) = _prep(inputs)
    key = (tc_tokens, has_b, has_aff)
    if key not in _CACHE:
        _CACHE[key] = build(*key)
    nc = _CACHE[key]
    res = run_bass_kernel_spmd(nc, in_maps, core_ids=list(range(NCORES)),
                               trace=True)
    out = np.concatenate([res.results[i]["out"] for i in range(NCORES)], axis=0)
    return out.reshape(B, S, H).astype(np.float32), res


# revision 56
# speedup vs baseline: 1.0441x; 1.0105x over previous
"""Trainium2 Bass kernel for nn_Adapter (per-token candidate attention + MLP + LN).

Data-parallel over tokens across 8 NeuronCores. Matmuls run in bf16 (fp32 PSUM
accumulation); softmax / attention-combine / layernorm run in fp32 on DVE/ACT.

Per-core pipeline (Tc = T/8 tokens, t-tiles of 128):
  - x, c streamed in; cast to bf16 via SWDGE DMA; PE-transposed to K-major.
  - q/k/v projections: PE matmuls, activations-stationary, weights moving.
  - scores: DVE mult + grouped reduce (contraction d=64, per candidate).
  - masked softmax over N=8 (no max-subtract needed; scores are O(1)).
  - ctx: DVE e-weighted accumulation of v from PSUM.
  - MLP: transposed-layout bf16 matmuls (weights stationary), gelu on ACT.
  - residual + layernorm fused on DVE/ACT; output DMA in natural layout.
"""

import numpy as np
import ml_dtypes

import concourse.bass as bass
import concourse.mybir as mybir
import concourse.tile as tile
from concourse.bass_utils import run_bass_kernel_spmd
from concourse.masks import make_identity

F32 = mybir.dt.float32
BF16 = mybir.dt.bfloat16
AX = mybir.AxisListType.X
ALU = mybir.AluOpType
ACTF = mybir.ActivationFunctionType

HID = 768
NH = 12
HD = 64
NCAND = 8
NCORES = 8
EPS = 1e-12
NI = HID // 128          # 6 input-feature chunks
NJ4 = 4 * HID // 128     # 24 hidden chunks
ESCALE = 1.0 / np.sqrt(HD)

_CACHE = {}


def _split_excess_waits(nc, max_waits=1):
    """walrus in this container only packs ONE sync-wait per ISA instruction;
    move excess on_wait entries onto inserted same-engine Drain carriers."""
    for func in nc.m.functions:
        for block in func.blocks:
            new = []
            for inst in block.instructions:
                si = inst.sync_info
                if si is not None:
                    waits = list(si.on_wait)
                    if len(waits) > max_waits:
                        k = 0
                        while len(waits) > max_waits:
                            chunk, waits = waits[:max_waits], waits[max_waits:]
                            carrier = mybir.InstNoOp(
                                name=f"{inst.name}-ws{k}", engine=inst.engine,
                                sync_info=mybir.SyncInfo(on_wait=chunk,
                                                         on_update=[]))
                            nc.register_instruction(carrier, overwrite=True)
                            new.append(carrier)
                            k += 1
                        inst.sync_info = mybir.SyncInfo(
                            on_wait=waits, on_update=list(si.on_update))
                new.append(inst)
            block.instructions = new


def build(tc_tokens, has_b, has_aff):
    """Build the single-core Bass graph (same graph runs SPMD on all cores)."""
    nt = tc_tokens // 128
    nc = bass.Bass()

    id_d = nc.dram_tensor("idb", [128, 128], BF16, kind="ExternalInput")
    idf_d = nc.dram_tensor("idf", [128, 128], F32, kind="ExternalInput")
    x_d = nc.dram_tensor("x", [tc_tokens, HID], F32, kind="ExternalInput")
    c_d = nc.dram_tensor("c", [tc_tokens, NCAND, HID], F32, kind="ExternalInput")
    m_d = nc.dram_tensor("m", [tc_tokens, NCAND], F32, kind="ExternalInput")
    wq_d = nc.dram_tensor("wq", [HID, HID], BF16, kind="ExternalInput")
    wk_d = nc.dram_tensor("wk", [HID, HID], BF16, kind="ExternalInput")
    wv_d = nc.dram_tensor("wv", [HID, HID], BF16, kind="ExternalInput")
    wt_d = nc.dram_tensor("wt", [HID, 4 * HID], BF16, kind="ExternalInput")
    wc_d = nc.dram_tensor("wc", [4 * HID, HID], BF16, kind="ExternalInput")
    if has_b:
        bq_d = nc.dram_tensor("bq", [HID], F32, kind="ExternalInput")
        bk_d = nc.dram_tensor("bk", [HID], F32, kind="ExternalInput")
        bv_d = nc.dram_tensor("bv", [HID], F32, kind="ExternalInput")
        bt_d = nc.dram_tensor("bt", [4 * HID], F32, kind="ExternalInput")
        bc_d = nc.dram_tensor("bc", [HID], F32, kind="ExternalInput")
    if has_aff:
        ga_d = nc.dram_tensor("ga", [HID], F32, kind="ExternalInput")
        be_d = nc.dram_tensor("be", [HID], F32, kind="ExternalInput")
    o_d = nc.dram_tensor("out", [tc_tokens, HID], F32, kind="ExternalOutput")

    with tile.TileContext(nc) as tc:
        consts = tc.alloc_tile_pool(name="consts", bufs=1)
        wpool = tc.alloc_tile_pool(name="wpool", bufs=1)
        mlpw = tc.alloc_tile_pool(name="mlpw", bufs=3)
        stage = tc.alloc_tile_pool(name="stage", bufs=6)
        xp = tc.alloc_tile_pool(name="xp", bufs=2)
        ctp = tc.alloc_tile_pool(name="ctp", bufs=2)
        qp = tc.alloc_tile_pool(name="qp", bufs=2)
        attn = tc.alloc_tile_pool(name="attn", bufs=2)
        sm = tc.alloc_tile_pool(name="sm", bufs=4)
        chk = tc.alloc_tile_pool(name="chk", bufs=2)
        h1p = tc.alloc_tile_pool(name="h1p", bufs=1)
        lnp = tc.alloc_tile_pool(name="lnp", bufs=2)

        ps_tr = tc.alloc_tile_pool(name="ps_tr", bufs=2, space="PSUM")
        ps_big = tc.alloc_tile_pool(name="ps_big", bufs=2, space="PSUM")
        ps_mlp = tc.alloc_tile_pool(name="ps_mlp", bufs=2, space="PSUM")

        ident_b = consts.tile([128, 128], BF16)
        nc.sync.dma_start(out=ident_b, in_=id_d[:, :])
        ident_f = consts.tile([128, 128], F32)
        nc.sync.dma_start(out=ident_f, in_=idf_d[:, :])
        m_all = consts.tile([128, nt, NCAND], F32)
        nc.sync.dma_start(out=m_all,
                          in_=m_d.rearrange("(t p) n -> p t n", p=128))
        c1e10 = consts.tile([128, 1], F32)
        nc.vector.memset(c1e10, 1e-10)
        ceps = consts.tile([128, 1], F32)
        nc.vector.memset(ceps, EPS)

        # resident qkv weights, pre-transposed on host: [in_i, out_j]
        wq_sb = wpool.tile([128, NI, HID], BF16)
        nc.sync.dma_start(out=wq_sb, in_=wq_d.rearrange("(c p) j -> p c j", p=128))
        wk_sb = wpool.tile([128, NI, HID], BF16)
        nc.sync.dma_start(out=wk_sb, in_=wk_d.rearrange("(c p) j -> p c j", p=128))
        wv_sb = wpool.tile([128, NI, HID], BF16)
        nc.sync.dma_start(out=wv_sb, in_=wv_d.rearrange("(c p) j -> p c j", p=128))

        if has_b:
            bq_rep = consts.tile([128, HID], F32)
            nc.gpsimd.dma_start(out=bq_rep, in_=bq_d.to_broadcast([128, HID]))
            bk_rep = consts.tile([128, HID], F32)
            nc.gpsimd.dma_start(out=bk_rep, in_=bk_d.to_broadcast([128, HID]))
            bv_rep = consts.tile([128, HID], F32)
            nc.gpsimd.dma_start(out=bv_rep, in_=bv_d.to_broadcast([128, HID]))
            bt_sb = consts.tile([128, NJ4], F32)
            nc.sync.dma_start(out=bt_sb, in_=bt_d.rearrange("(c p) -> p c", p=128))
            bc_sb = consts.tile([128, NI], F32)
            nc.sync.dma_start(out=bc_sb, in_=bc_d.rearrange("(c p) -> p c", p=128))
        if has_aff:
            ga_rep = consts.tile([128, HID], F32)
            nc.gpsimd.dma_start(out=ga_rep, in_=ga_d.to_broadcast([128, HID]))
            be_rep = consts.tile([128, HID], F32)
            nc.gpsimd.dma_start(out=be_rep, in_=be_d.to_broadcast([128, HID]))

        drp = tc.alloc_tile_pool(name="drp", bufs=12, space="DRAM")

        # chunking for the MLP (groups of up to 4 t-tiles -> 512-token chunks)
        chunks = [list(range(s, min(s + 4, nt))) for s in range(0, nt, 4)]

        def emit_ctx_transpose(ctxT_, tloc_, ctx_b_):
            ctxT_ps = ps_tr.tile([128, HID], BF16, tag="trps")
            for i in range(NI):
                nc.tensor.transpose(ctxT_ps[:, i * 128:(i + 1) * 128],
                                    ctx_b_[:, i * 128:(i + 1) * 128], ident_b)
            nc.scalar.copy(
                ctxT_[:, :, tloc_ * 128:(tloc_ + 1) * 128],
                ctxT_ps.rearrange("p (c j) -> p c j", c=NI))

        for chunk in chunks:
            cw = 128 * len(chunk)
            ctxT = chk.tile([128, NI, 512], BF16, tag="ctxT")
            pending_ctx = None

            for tloc, tt in enumerate(chunk):
                t0 = tt * 128
                # ---- load + transpose x (bf16, K-major) ----
                x_b = xp.tile([128, HID], BF16, tag="x_b")
                nc.gpsimd.dma_start(out=x_b, in_=x_d[t0:t0 + 128, :])  # cast f32->bf16
                m_t = m_all[:, tt, :]
                if pending_ctx is not None:
                    emit_ctx_transpose(ctxT, *pending_ctx)
                    pending_ctx = None

                xT_ps = ps_tr.tile([128, HID], BF16, tag="trps")
                for i in range(NI):
                    nc.tensor.transpose(xT_ps[:, i * 128:(i + 1) * 128],
                                        x_b[:, i * 128:(i + 1) * 128], ident_b)
                xT = xp.tile([128, NI, 128], BF16, tag="xT")
                nc.scalar.copy(xT, xT_ps.rearrange("p (c j) -> p c j", c=NI))

                # ---- q projection ----
                q_ps = ps_big.tile([128, HID], F32, tag="big")
                for i in range(NI):
                    nc.tensor.matmul(q_ps[:, :512], xT[:, i, :], wq_sb[:, i, :512],
                                     start=(i == 0), stop=(i == NI - 1))
                    nc.tensor.matmul(q_ps[:, 512:], xT[:, i, :], wq_sb[:, i, 512:],
                                     start=(i == 0), stop=(i == NI - 1))
                q_sb = qp.tile([128, HID], BF16, tag="q_sb")
                if has_b:
                    q_f = qp.tile([128, HID], F32, tag="q_f")
                    nc.scalar.copy(q_f, q_ps)
                    nc.vector.tensor_add(q_sb, q_f, bq_rep)
                else:
                    nc.scalar.copy(q_sb, q_ps)

                # ---- load c slices: cast to a bf16 DRAM bounce, then K-major
                # via the DMA xbar transpose (no PE/ACT involvement) ----
                cT = ctp.tile([128, NCAND * NI, 128], BF16, tag="cT")
                for n in range(NCAND):
                    c_bf = drp.tile([128, HID], BF16, tag="c_bf")
                    nc.gpsimd.dma_start(out=c_bf, in_=c_d[t0:t0 + 128, n, :])
                    for i in range(NI):
                        nc.sync.dma_start_transpose(
                            out=cT[:, n * NI + i, :],
                            in_=c_bf[:, i * 128:(i + 1) * 128])

                # ---- scores: k_n projection + q.k reduce ----
                scores = attn.tile([128, NH, NCAND], F32, tag="scores")
                for n in range(NCAND):
                    k_ps = ps_big.tile([128, HID], F32, tag="big")
                    for i in range(NI):
                        lhsT = cT[:, n * NI + i, :]
                        nc.tensor.matmul(k_ps[:, :512], lhsT, wk_sb[:, i, :512],
                                         start=(i == 0), stop=(i == NI - 1))
                        nc.tensor.matmul(k_ps[:, 512:], lhsT, wk_sb[:, i, 512:],
                                         start=(i == 0), stop=(i == NI - 1))
                    prod = attn.tile([128, HID], F32, tag="prod")
                    nc.vector.tensor_mul(prod, k_ps, q_sb)
                    nc.vector.tensor_reduce(
                        out=scores[:, :, n:n + 1],
                        in_=prod.rearrange("p (h d) -> p h d", h=NH),
                        axis=AX, op=ALU.add)
                if has_b:
                    # scores += sum_d q[t,h,d]*bk[h,d]  (constant across n)
                    prod = attn.tile([128, HID], F32, tag="prod")
                    nc.vector.tensor_mul(prod, q_sb, bk_rep)
                    qbk = sm.tile([128, NH, 1], F32, tag="qbk")
                    nc.vector.tensor_reduce(
                        out=qbk, in_=prod.rearrange("p (h d) -> p h d", h=NH),
                        axis=AX, op=ALU.add)
                    nc.vector.tensor_add(scores, scores,
                                         qbk.broadcast_to([128, NH, NCAND]))

                # ---- masked softmax over n (no max-subtract; scores are O(1)) ----
                # masked scores -> 0 (exp(0)=1.0 == exp(1e-10*scale) in fp32)
                om = sm.tile([128, NCAND], F32, tag="om")
                nc.vector.tensor_scalar(out=om, in0=m_t, scalar1=-1.0, scalar2=1.0,
                                        op0=ALU.mult, op1=ALU.add)
                nc.vector.tensor_mul(scores, scores,
                                     om.unsqueeze(1).broadcast_to([128, NH, NCAND]))
                e_t = attn.tile([128, NH, NCAND], F32, tag="e_t")
                nc.scalar.activation(e_t, scores, ACTF.Exp, scale=ESCALE)
                esum = sm.tile([128, NH], F32, tag="esum")
                nc.vector.tensor_reduce(out=esum, in_=e_t, axis=AX, op=ALU.add)
                recip = sm.tile([128, NH], F32, tag="recip")
                nc.vector.reciprocal(recip, esum)
                msum = sm.tile([128, 1], F32, tag="msum")
                nc.vector.tensor_reduce(out=msum, in_=m_t, axis=AX, op=ALU.add)
                notall = sm.tile([128, 1], F32, tag="notall")
                nc.vector.tensor_scalar(out=notall, in0=msum, scalar1=float(NCAND) - 0.5,
                                        scalar2=None, op0=ALU.is_lt)
                nc.vector.tensor_scalar(out=recip, in0=recip, scalar1=notall,
                                        scalar2=None, op0=ALU.mult)
                # pre-scale e by 1/esum (and the all-masked zeroing) so the
                # v-side accumulation needs no final normalization pass
                nc.vector.tensor_mul(e_t, e_t,
                                     recip.unsqueeze(2).broadcast_to([128, NH, NCAND]))

                # ---- ctx: v_n projection + attn-weighted accumulation ----
                ctx = attn.tile([128, NH, HD], F32, tag="ctx")
                ctx_b = attn.tile([128, HID], BF16, tag="ctx_b", bufs=5)
                ctx_b3 = ctx_b.rearrange("p (h d) -> p h d", h=NH)
                for n in range(NCAND):
                    v_ps = ps_big.tile([128, HID], F32, tag="big")
                    for i in range(NI):
                        lhsT = cT[:, n * NI + i, :]
                        nc.tensor.matmul(v_ps[:, :512], lhsT, wv_sb[:, i, :512],
                                         start=(i == 0), stop=(i == NI - 1))
                        nc.tensor.matmul(v_ps[:, 512:], lhsT, wv_sb[:, i, 512:],
                                         start=(i == 0), stop=(i == NI - 1))
                    e_b = e_t[:, :, n:n + 1].broadcast_to([128, NH, HD])
                    v3 = v_ps.rearrange("p (h d) -> p h d", h=NH)
                    if n == 0:
                        nc.vector.tensor_mul(ctx, v3, e_b)
                    else:
                        prodv = attn.tile([128, NH, HD], F32, tag="prodv")
                        nc.vector.tensor_mul(prodv, v3, e_b)
                        last = (n == NCAND - 1) and not has_b
                        nc.vector.tensor_add(ctx_b3 if last else ctx, ctx, prodv)
                if has_b:
                    # ctx += bv * notall (softmax weights sum to 1)
                    nc.vector.scalar_tensor_tensor(
                        out=ctx_b3, in0=bv_rep.rearrange("p (h d) -> p h d", h=NH),
                        scalar=notall, in1=ctx, op0=ALU.mult, op1=ALU.add)
                pending_ctx = (tloc, ctx_b)

            emit_ctx_transpose(ctxT, *pending_ctx)

            # ---- MLP over the chunk (transposed layout) ----
            h1T = h1p.tile([128, NJ4, 512], BF16, tag="h1T")
            for j in range(NJ4):
                wt_t = mlpw.tile([128, NI, 128], BF16, tag="wt_t", bufs=6)
                nc.sync.dma_start(
                    out=wt_t,
                    in_=wt_d[:, j * 128:(j + 1) * 128].rearrange(
                        "(c p) j -> p c j", p=128))
                h1_ps = ps_mlp.tile([128, 512], F32, tag="mlpps")
                for i in range(NI):
                    nc.tensor.matmul(h1_ps[:, :cw], wt_t[:, i, :], ctxT[:, i, :cw],
                                     start=(i == 0), stop=(i == NI - 1))
                nc.scalar.activation(h1T[:, j, :cw], h1_ps[:, :cw], ACTF.Gelu,
                                     bias=(bt_sb[:, j:j + 1] if has_b else 0.0))

            o2T = chk.tile([128, NI, 512], F32, tag="o2T")
            for o in range(NI):
                wc_t = mlpw.tile([128, NJ4, 128], BF16, tag="wc_t")
                nc.sync.dma_start(
                    out=wc_t,
                    in_=wc_d[:, o * 128:(o + 1) * 128].rearrange(
                        "(c p) j -> p c j", p=128))
                o2_ps = ps_mlp.tile([128, 512], F32, tag="mlpps")
                for j in range(NJ4):
                    nc.tensor.matmul(o2_ps[:, :cw], wc_t[:, j, :], h1T[:, j, :cw],
                                     start=(j == 0), stop=(j == NJ4 - 1))
                nc.scalar.activation(o2T[:, o, :cw], o2_ps[:, :cw], ACTF.Copy,
                                     bias=(bc_sb[:, o:o + 1] if has_b else 0.0))

            # ---- back to natural layout + residual + layernorm ----
            for tloc, tt in enumerate(chunk):
                t0 = tt * 128
                o2n_ps = ps_big.tile([128, HID], F32, tag="big")
                for o in range(NI):
                    nc.tensor.transpose(o2n_ps[:, o * 128:(o + 1) * 128],
                                        o2T[:, o, tloc * 128:(tloc + 1) * 128],
                                        ident_f)
                x_f = lnp.tile([128, HID], F32, tag="x_f")
                nc.sync.dma_start(out=x_f, in_=x_d[t0:t0 + 128, :])

                y_sb = lnp.tile([128, HID], F32, tag="y_sb")
                sums = sm.tile([128, 1], F32, tag="sums")
                nc.vector.scalar_tensor_tensor(
                    out=y_sb, in0=o2n_ps, scalar=1.0, in1=x_f,
                    op0=ALU.mult, op1=ALU.add, accum_out=sums)
                out_sb = lnp.tile([128, HID], F32, tag="out_sb")
                sumsq = sm.tile([128, 1], F32, tag="sumsq")
                nc.vector.scalar_tensor_tensor(
                    out=out_sb, in0=y_sb, scalar=1.0, in1=y_sb,
                    op0=ALU.mult, op1=ALU.mult, accum_out=sumsq)
                mean = sm.tile([128, 1], F32, tag="mean")
                nc.vector.tensor_scalar(out=mean, in0=sums, scalar1=1.0 / HID,
                                        scalar2=None, op0=ALU.mult)
                msq = sm.tile([128, 1], F32, tag="msq")
                nc.vector.tensor_mul(msq, mean, mean)
                var = sm.tile([128, 1], F32, tag="var")
                nc.vector.tensor_scalar(out=var, in0=sumsq, scalar1=1.0 / HID,
                                        scalar2=msq, op0=ALU.mult, op1=ALU.subtract)
                # rstd = exp(-0.5 * ln(var + eps)) — Ln/Exp share one ACT table set
                lnv = sm.tile([128, 1], F32, tag="lnv")
                nc.scalar.activation(lnv, var, ACTF.Ln, bias=ceps)
                rstd = sm.tile([128, 1], F32, tag="rstd")
                nc.scalar.activation(rstd, lnv, ACTF.Exp, scale=-0.5)

                nc.vector.tensor_scalar(out=out_sb, in0=y_sb, scalar1=mean,
                                        scalar2=rstd, op0=ALU.subtract, op1=ALU.mult)
                if has_aff:
                    nc.vector.tensor_mul(out_sb, out_sb, ga_rep)
                    nc.vector.tensor_add(out_sb, out_sb, be_rep)
                nc.sync.dma_start(out=o_d[t0:t0 + 128, :], in_=out_sb)

        for p in reversed((consts, wpool, mlpw, stage, xp, ctp, qp, attn, sm,
                           chk, h1p, lnp, ps_tr, ps_big, ps_mlp)):
            p.release()
    _split_excess_waits(nc)
    return nc


def _prep(inputs):
    ins = {k: np.asarray(v) for k, v in inputs.items()}
    x = ins["layer_output"].astype(np.float32)
    c = ins["candidates_embeddings"].astype(np.float32)
    m = ins["candidates_mask"].astype(np.float32)
    B, S, H = x.shape
    T = B * S
    n_ = c.shape[2]
    assert H == HID and n_ == NCAND and T % (NCORES * 128) == 0

    has_b = any(np.any(ins[k] != 0) for k in ("bq", "bk", "bv", "bt", "bc"))
    has_aff = bool(np.any(ins["gamma"] != 1) or np.any(ins["beta"] != 0))

    bf = ml_dtypes.bfloat16
    weights = {
        "idb": np.eye(128, dtype=np.float32).astype(bf),
        "idf": np.eye(128, dtype=np.float32),
        "wq": np.ascontiguousarray(ins["Wq"].astype(np.float32).T).astype(bf),
        "wk": np.ascontiguousarray(ins["Wk"].astype(np.float32).T).astype(bf),
        "wv": np.ascontiguousarray(ins["Wv"].astype(np.float32).T).astype(bf),
        "wt": np.ascontiguousarray(ins["Wt"].astype(np.float32).T).astype(bf),
        "wc": np.ascontiguousarray(ins["Wc"].astype(np.float32).T).astype(bf),
    }
    if has_b:
        for k_src, k_dst in (("bq", "bq"), ("bk", "bk"), ("bv", "bv"),
                             ("bt", "bt"), ("bc", "bc")):
            weights[k_dst] = ins[k_src].astype(np.float32)
    if has_aff:
        weights["ga"] = ins["gamma"].astype(np.float32)
        weights["be"] = ins["beta"].astype(np.float32)

    tc_tokens = T // NCORES
    xf = x.reshape(T, H)
    cf = c.reshape(T, NCAND, H)
    mf = m.reshape(T, NCAND)
    in_maps = []
    for k in range(NCORES):
        sl = slice(k * tc_tokens, (k + 1) * tc_tokens)
        im = {"x": np.ascontiguousarray(xf[sl]),
              "c": np.ascontiguousarray(cf[sl]),
              "m": np.ascontiguousarray(mf[sl])}
        im.update(weights)
        in_maps.append(im)
    return in_maps, tc_tokens, has_b, has_aff, (B, S, H)


def kernel(**inputs):
    in_maps, tc_tokens, has_b, has_aff, (B, S, H) = _prep(inputs)
    key = (tc_tokens, has_b, has_aff)
    if key not in _CACHE:
        _CACHE[key] = build(*key)
    nc = _CACHE[key]
    res = run_bass_kernel_spmd(nc, in_maps, core_ids=list(range(NCORES)))
    out = np.concatenate([res.results[i]["out"] for i in range(NCORES)], axis=0)
    return out.reshape(B, S, H).astype(np.float32)


# exposed for test.py profiling
def kernel_profiled(**inputs):
    in_maps, tc_tokens, has_b, has_aff, (B, S, H) = _prep(inputs)
    key = (tc_tokens, has_b, has_aff)
    if key not in _CACHE:
        _CACHE[key] = build(*key)
    nc = _CACHE[key]
    res = run_bass_kernel_spmd(nc, in_maps, core_ids=list(range(NCORES)),
                               trace=True)
    out = np.concatenate([res.results[i]["out"] for i in range(NCORES)], axis=0)
    return out.reshape(B, S, H).astype(np.float32), res
